# revision 20
# baseline (speedup 1.0000x reference)
"""AngleLoss distributed Trainium2 kernel.

mean(arccos(dot(o,t) / (|o||t|))) over 2,097,152 rows of 3-vectors,
data-parallel over 8 NeuronCores.

Math (per row, division/sign-free):
    prod = (sum o^2) * (sum t^2)
    c    = dot * absrsqrt(prod)            # = cos(theta)
    num  = relu(1 - c)                     # clamped 1-c
    r2   = absrsqrt(|1 - c^2|)
    g    = num * r2                        # = sqrt((1-c)/(1+c)) = tan(theta/2)
    theta = 2 * arctan(g)                  # arctan table covers [0, inf)
Per-core output: [128,1] f32 partial sums of arctan(g); host computes
mean = 2 * total / N.
"""

import sys
import numpy as np

if "/opt/trn_rl_repo" not in sys.path:
    sys.path.insert(0, "/opt/trn_rl_repo")

N_CORES = 8
R_TOTAL = 256 * 8192  # 2097152 rows
PER_CORE = R_TOTAL // N_CORES  # 262144
P = 128
FREE = PER_CORE // P  # 2048

# Tunables
N_TILES = 4
# planes whose square runs on VectorE (f32 tensor_tensor) instead of ScalarE
SQ_ON_VE = ()
USE_RAW = True
N_INBUF = 4
TILE_SIZES = (128, 256, 384, 512, 512, 256)
assert sum(TILE_SIZES) == FREE

_BUILD_CACHE = {}


def _build_nc_raw():
    key = ("raw", TILE_SIZES, N_INBUF)
    if key in _BUILD_CACHE:
        return _BUILD_CACHE[key]

    import concourse.bass as bass
    from concourse import bacc, mybir

    AF = mybir.ActivationFunctionType
    OP = mybir.AluOpType
    f32 = mybir.dt.float32
    bf16 = mybir.dt.bfloat16

    sizes = list(TILE_SIZES)
    T = len(sizes)
    NB = N_INBUF
    Fmax = max(sizes)
    offs = [0]
    for s in sizes:
        offs.append(offs[-1] + s)

    nc = bacc.Bacc(
        "TRN2", target_bir_lowering=False, debug=False, num_devices=N_CORES
    )
    # tile-major flat layout: tile i occupies 6*P*F_i floats; within a tile,
    # partition p's 6*F_i floats are contiguous (one DMA descriptor per row).
    x = nc.dram_tensor("x", [6 * P * FREE], f32, kind="ExternalInput")
    out = nc.dram_tensor("out", [P, 32], f32, kind="ExternalOutput")
    xf = x.ap()

    def sb(name, shape, dtype):
        return nc.alloc_sbuf_tensor(name, list(shape), dtype).ap()

    inbuf = [sb(f"inb{b}", [P, 6 * Fmax], f32) for b in range(NB)]
    sqb = [sb(f"sqb{b}", [P, 6 * Fmax], bf16) for b in range(2)]
    m = sb("m", [P, 3 * Fmax], bf16)
    dxy = sb("dxy", [P, Fmax], bf16)
    dotb = [sb(f"dot{b}", [P, Fmax], bf16) for b in range(2)]
    pair = sb("pair", [P, 2 * Fmax], bf16)  # [oo1, tt1]
    oott = sb("oott", [P, 2 * Fmax], bf16)  # [oo, tt]
    prodb = [sb(f"prod{b}", [P, Fmax], bf16) for b in range(2)]
    cb = sb("c", [P, Fmax], bf16)
    c2b = [sb(f"c2{b}", [P, Fmax], bf16) for b in range(2)]
    numpb = [sb(f"nump{b}", [P, Fmax], bf16) for b in range(2)]
    r1b = [sb(f"r1{b}", [P, Fmax], bf16) for b in range(2)]
    r2b = [sb(f"r2{b}", [P, Fmax], bf16) for b in range(2)]
    g_all = sb("g_all", [P, FREE], bf16)
    t_scr = sb("t_scr", [P, FREE], bf16)
    asum = sb("asum", [P, 32], f32)
    warm = sb("warm", [P, 1], bf16)
    bias0 = sb("bias0", [P, 1], f32)
    bias1 = sb("bias1", [P, 1], f32)

    S_fin = nc.alloc_semaphore("s_fin")
    NQ = 4  # rotating per-tile DMA sems: exact per-tile completion
    S_dmaq = [nc.alloc_semaphore(f"s_dma{q}") for q in range(NQ)]
    S_dmo = nc.alloc_semaphore("s_dmo")
    S_bias = nc.alloc_semaphore("s_bias")

    def dma_wait(eng, i):
        eng.wait_ge(S_dmaq[i % NQ], 32 * (i // NQ + 1))
    S_vein = nc.alloc_semaphore("s_vein")  # bigmult done (input o,t read)
    S_prod = nc.alloc_semaphore("s_prod")
    S_c2 = nc.alloc_semaphore("s_c2")
    S_veg = nc.alloc_semaphore("s_veg")
    S_sq = nc.alloc_semaphore("s_sq")
    S_r1 = nc.alloc_semaphore("s_r1")
    S_r2 = nc.alloc_semaphore("s_r2")

    with nc.Block(no_gpsimd_drain=True) as block:

        # each tile's load is split in 2 column-slices, one per HWDGE ring
        # (sync + scalar) -> ~2x per-tile DMA rate vs a single ring.
        # Tile i is complete when its sem slot reaches 32*(i//NQ+1).
        def issue_chunk(eng, i, k):
            F = sizes[i]
            tile = xf[6 * P * offs[i] : 6 * P * offs[i + 1]].rearrange(
                "(p f) -> p f", p=P
            )
            eng.dma_start(
                out=inbuf[i % NB][:, 3 * k * F : 3 * (k + 1) * F],
                in_=tile[:, 3 * k * F : 3 * (k + 1) * F],
            ).then_inc(S_dmaq[i % NQ], 16)

        def issue_guard(eng, i):
            if i >= NB:
                eng.wait_ge(S_vein, i - NB + 1)
                eng.wait_ge(S_sq, i - NB + 1)

        @block.sync
        def _(sync):
            for i in range(T):
                issue_guard(sync, i)
                issue_chunk(sync, i, 0)
            sync.wait_ge(S_fin, 1)
            sync.dma_start(out=out.ap()[:, :], in_=asum[:, :]).then_inc(S_dmo, 16)
            sync.wait_ge(S_dmo, 16)



        # Software pipeline with lag: VE iter i does the front half of tile
        # i, then c/c2/nump of tile i-1 and g of tile i-2, so in steady state
        # it never waits on same-iteration ScalarE results.
        @block.vector
        def _(vector):
            vector.memset(bias0[:], 0.0).then_inc(S_bias)
            vector.memset(bias1[:], 1.0).then_inc(S_bias)
            for i in range(T + 2):
                h = i % 2
                hp = (i - 1) % 2
                hg = (i - 2) % 2
                if i < T:
                    F = sizes[i]
                    inb = inbuf[i % NB]
                    dma_wait(vector, i)
                    vector.tensor_tensor(
                        m[:, : 3 * F], inb[:, : 3 * F], inb[:, 3 * F : 6 * F],
                        OP.mult,
                    ).then_inc(S_vein)
                    vector.tensor_tensor(
                        dxy[:, :F], m[:, :F], m[:, F : 2 * F], OP.add
                    )
                    vector.tensor_tensor(
                        dotb[h][:, :F], dxy[:, :F], m[:, 2 * F : 3 * F], OP.add
                    )
                    vector.wait_ge(S_sq, i + 1)
                    sq6 = sqb[h][:, : 6 * F].rearrange("p (j f) -> p j f", j=6)
                    pr = pair[:, : 2 * F].rearrange("p (j f) -> p j f", j=2)
                    ot = oott[:, : 2 * F].rearrange("p (j f) -> p j f", j=2)
                    vector.tensor_tensor(
                        pr[:], sq6[:, 0:5:3, :], sq6[:, 1:6:3, :], OP.add
                    )
                    vector.tensor_tensor(ot[:], pr[:], sq6[:, 2:6:3, :], OP.add)
                    vector.tensor_tensor(
                        prodb[h][:, :F], ot[:, 0, :], ot[:, 1, :], OP.mult
                    ).then_inc(S_prod)
                if 1 <= i <= T:
                    F = sizes[i - 1]
                    vector.wait_ge(S_r1, i)
                    vector.tensor_tensor(
                        cb[:, :F], dotb[hp][:, :F], r1b[hp][:, :F], OP.mult
                    )
                    vector.tensor_tensor(
                        c2b[hp][:, :F], cb[:, :F], cb[:, :F], OP.mult
                    ).then_inc(S_c2)
                    # nump = min(c-1, 0) = -relu(1-c)
                    vector.tensor_scalar(
                        numpb[hp][:, :F], cb[:, :F], 1.0, 0.0, OP.subtract, OP.min
                    )
                if i >= 2:
                    F = sizes[i - 2]
                    vector.wait_ge(S_r2, i - 1)
                    vector.tensor_tensor(
                        g_all[:, offs[i - 2] : offs[i - 1]],
                        numpb[hg][:, :F], r2b[hg][:, :F], OP.mult,
                    ).then_inc(S_veg)

        @block.scalar
        def _(scalar):
            for j in range(min(NB, T)):
                issue_chunk(scalar, j, 1)
            # pin the abs_reciprocal_sqrt table set before any real work
            # (first activation in program order decides the first table
            # load; bias=warm itself avoids needing an initialized const)
            scalar.activation(
                warm[:], warm[:], AF.Abs_reciprocal_sqrt, bias=warm[:], scale=0.0
            )
            scalar.wait_ge(S_bias, 2)
            scalar.wait_ge(S_dmaq[0], 16)
            scalar.activation(
                sqb[0][:, : 6 * sizes[0]], inbuf[0][:, : 6 * sizes[0]], AF.Square,
                bias=bias0[:],
            ).then_inc(S_sq)
            for i in range(T):
                h = i % 2
                hp = (i - 1) % 2
                if i + 1 < T:
                    # issue sq[i+1] before r1[i] so VE's pair-adds for tile
                    # i+1 are never starved behind this iteration's r1/r2
                    hn = (i + 1) % 2
                    F1 = sizes[i + 1]
                    dma_wait(scalar, i + 1)
                    if i + 1 >= 2:
                        scalar.wait_ge(S_prod, i)
                    scalar.activation(
                        sqb[hn][:, : 6 * F1], inbuf[(i + 1) % NB][:, : 6 * F1],
                        AF.Square, bias=bias0[:],
                    ).then_inc(S_sq)
                F = sizes[i]
                scalar.wait_ge(S_prod, i + 1)
                scalar.activation(
                    r1b[h][:, :F], prodb[h][:, :F], AF.Abs_reciprocal_sqrt,
                    bias=bias0[:],
                ).then_inc(S_r1)
                if i + NB < T:
                    # consumers of tile i are done (implied by S_prod>=i+1
                    # and ScalarE program order), so inbuf[(i+NB)%NB] is free
                    issue_chunk(scalar, i + NB, 1)
                if i >= 1:
                    Fp = sizes[i - 1]
                    scalar.wait_ge(S_c2, i)
                    scalar.activation(
                        r2b[hp][:, :Fp], c2b[hp][:, :Fp], AF.Abs_reciprocal_sqrt,
                        bias=bias1[:], scale=-1.0,
                    ).then_inc(S_r2)
            hp = (T - 1) % 2
            Fp = sizes[T - 1]
            scalar.wait_ge(S_c2, T)
            scalar.activation(
                r2b[hp][:, :Fp], c2b[hp][:, :Fp], AF.Abs_reciprocal_sqrt,
                bias=bias1[:], scale=-1.0,
            ).then_inc(S_r2)
            # dummy arctan: forces the sigmoid-set table load now, overlapped
            # with VE's final g multiplies
            scalar.activation(warm[:], warm[:], AF.Arctan, bias=bias0[:], scale=0.0)
            scalar.wait_ge(S_veg, T)
            scalar.activation(
                t_scr[:], g_all[:], AF.Arctan, bias=bias0[:],
                accum_out=asum[:, 0:1],
            ).then_inc(S_fin)

    nc.compile()
    _BUILD_CACHE[key] = nc
    return nc


def _build_nc():
    key = (N_TILES, tuple(SQ_ON_VE))
    if key in _BUILD_CACHE:
        return _BUILD_CACHE[key]

    from contextlib import ExitStack

    import concourse.bass as bass
    import concourse.tile as tile
    from concourse import bacc, mybir

    AF = mybir.ActivationFunctionType
    OP = mybir.AluOpType
    f32 = mybir.dt.float32
    bf16 = mybir.dt.bfloat16

    Ft = FREE // N_TILES

    nc = bacc.Bacc(
        "TRN2", target_bir_lowering=False, debug=False, num_devices=N_CORES
    )
    x = nc.dram_tensor("x", [6 * P, FREE], f32, kind="ExternalInput")
    out = nc.dram_tensor("out", [P, 1], f32, kind="ExternalOutput")

    with tile.TileContext(nc) as tc, ExitStack() as ctx:
        inp = ctx.enter_context(tc.tile_pool(name="inp", bufs=2))
        mid = ctx.enter_context(tc.tile_pool(name="mid", bufs=2))
        per = ctx.enter_context(tc.tile_pool(name="persist", bufs=1))

        g_all = per.tile([P, FREE], bf16)
        t_scr = per.tile([P, FREE], bf16)
        asum = per.tile([P, 1], f32)

        # [6*P, FREE] -> [P, 6, FREE]: partition-stride FREE, plane-stride P*FREE
        xa = x.ap().rearrange("(j p) f -> p j f", j=6)

        for i in range(N_TILES):
            sl = bass.ts(i, Ft)

            blk = inp.tile([P, 6, Ft], f32, tag="inblk")
            nc.sync.dma_start(blk[:], xa[:, :, sl])
            planes = [blk[:, j, :] for j in range(6)]
            ox, oy, oz, tx, ty, tz = planes

            def tt(name, a, b, op, dtype=bf16):
                t = mid.tile([P, Ft], dtype, tag=name)
                nc.vector.tensor_tensor(t[:], a[:], b[:], op)
                return t

            # dot = ox*tx + oy*ty + oz*tz  (mults f32->bf16, adds bf16)
            mx = tt("mx", ox, tx, OP.mult)
            my = tt("my", oy, ty, OP.mult)
            mz = tt("mz", oz, tz, OP.mult)
            dxy = tt("dxy", mx, my, OP.add)
            dot = tt("dot", dxy, mz, OP.add)

            # squares -> oo, tt
            sq = {}
            for name, pl in (
                ("xo", ox), ("yo", oy), ("zo", oz),
                ("xt", tx), ("yt", ty), ("zt", tz),
            ):
                s = mid.tile([P, Ft], bf16, tag="sq" + name)
                if name in SQ_ON_VE:
                    nc.vector.tensor_tensor(s[:], pl[:], pl[:], OP.mult)
                else:
                    nc.scalar.square(s[:], pl[:])
                sq[name] = s
            oo1 = tt("oo1", sq["xo"], sq["yo"], OP.add)
            oo = tt("oo", oo1, sq["zo"], OP.add)
            tt1 = tt("tt1", sq["xt"], sq["yt"], OP.add)
            ttn = tt("ttn", tt1, sq["zt"], OP.add)

            prod = tt("prod", oo, ttn, OP.mult)
            r1 = mid.tile([P, Ft], bf16, tag="r1")
            nc.scalar.activation(r1[:], prod[:], AF.Abs_reciprocal_sqrt)
            c = tt("c", dot, r1, OP.mult)
            c2 = tt("c2", c, c, OP.mult)
            # num_neg = min(c-1, 0) = -relu(1-c); sign is undone on the host
            nump = mid.tile([P, Ft], bf16, tag="nump")
            nc.vector.tensor_scalar(
                nump[:], c[:], 1.0, 0.0, OP.subtract, OP.min
            )
            r2 = mid.tile([P, Ft], bf16, tag="r2")
            nc.scalar.activation(
                r2[:], c2[:], AF.Abs_reciprocal_sqrt, bias=1.0, scale=-1.0
            )
            nc.vector.tensor_tensor(g_all[:, sl], nump[:], r2[:], OP.mult)

        # one arctan pass over the whole shard; accum_out = per-partition sum
        # (accumulates sum of arctan(-g) = -sum arctan(g))
        nc.scalar.activation(
            t_scr[:], g_all[:], AF.Arctan, accum_out=asum[:]
        )
        nc.sync.dma_start(out.ap()[:, :], asum[:])

    nc.compile()
    _BUILD_CACHE[key] = nc
    return nc


def _shard_inputs(outputs, targets):
    o = np.ascontiguousarray(np.asarray(outputs), dtype=np.float32).reshape(-1, 3)
    t = np.ascontiguousarray(np.asarray(targets), dtype=np.float32).reshape(-1, 3)
    T = N_TILES
    Ft = FREE // T
    in_maps = []
    for cidx in range(N_CORES):
        lo, hi = cidx * PER_CORE, (cidx + 1) * PER_CORE
        oc = o[lo:hi]
        tc_ = t[lo:hi]
        planes = np.empty((6, P, FREE), dtype=np.float32)
        for k in range(3):
            planes[k] = oc[:, k].reshape(P, FREE)
            planes[3 + k] = tc_[:, k].reshape(P, FREE)
        if USE_RAW:
            # tile-major flat: per tile i, [P, 6, F_i] row-contiguous
            blocks = []
            off = 0
            for F in TILE_SIZES:
                blk = planes[:, :, off : off + F]  # [6, P, F]
                blocks.append(
                    np.ascontiguousarray(blk.transpose(1, 0, 2)).reshape(-1)
                )
                off += F
            arr = np.concatenate(blocks)
        else:
            arr = planes.reshape(6 * P, FREE)
        in_maps.append({"x": arr})
    return in_maps


LAST_RESULT = None


def kernel(outputs, targets):
    global LAST_RESULT
    import os

    from concourse.bass_utils import run_bass_kernel_spmd

    nc = _build_nc_raw() if USE_RAW else _build_nc()
    in_maps = _shard_inputs(outputs, targets)
    trace = bool(os.environ.get("ANGLE_KERNEL_TRACE"))
    res = run_bass_kernel_spmd(
        nc, in_maps, core_ids=list(range(N_CORES)), trace=trace
    )
    LAST_RESULT = res
    total = 0.0
    for rmap in res.results:
        o = np.asarray(rmap["out"], dtype=np.float64)
        if USE_RAW:
            o = o[:, 0]  # cols 1.. are DMA padding
        total += o.sum()
    # device accumulates sum(arctan(-g)); theta = 2*arctan(g)
    mean = -2.0 * total / R_TOTAL
    return np.float32(mean)


# revision 21
# speedup vs baseline: 1.0001x; 1.0001x over previous
"""AngleLoss distributed Trainium2 kernel.

mean(arccos(dot(o,t) / (|o||t|))) over 2,097,152 rows of 3-vectors,
data-parallel over 8 NeuronCores.

Math (per row, division/sign-free):
    prod = (sum o^2) * (sum t^2)
    c    = dot * absrsqrt(prod)            # = cos(theta)
    num  = relu(1 - c)                     # clamped 1-c
    r2   = absrsqrt(|1 - c^2|)
    g    = num * r2                        # = sqrt((1-c)/(1+c)) = tan(theta/2)
    theta = 2 * arctan(g)                  # arctan table covers [0, inf)
Per-core output: [128,1] f32 partial sums of arctan(g); host computes
mean = 2 * total / N.
"""

import sys
import numpy as np

if "/opt/trn_rl_repo" not in sys.path:
    sys.path.insert(0, "/opt/trn_rl_repo")

N_CORES = 8
R_TOTAL = 256 * 8192  # 2097152 rows
PER_CORE = R_TOTAL // N_CORES  # 262144
P = 128
FREE = PER_CORE // P  # 2048

# Tunables
N_TILES = 4
# planes whose square runs on VectorE (f32 tensor_tensor) instead of ScalarE
SQ_ON_VE = ()
USE_RAW = True
N_INBUF = 4
TILE_SIZES = (128, 256, 384, 512, 512, 256)
assert sum(TILE_SIZES) == FREE

_BUILD_CACHE = {}


def _build_nc_raw():
    key = ("raw", TILE_SIZES, N_INBUF)
    if key in _BUILD_CACHE:
        return _BUILD_CACHE[key]

    import concourse.bass as bass
    from concourse import bacc, mybir

    AF = mybir.ActivationFunctionType
    OP = mybir.AluOpType
    f32 = mybir.dt.float32
    bf16 = mybir.dt.bfloat16

    sizes = list(TILE_SIZES)
    T = len(sizes)
    NB = N_INBUF
    Fmax = max(sizes)
    offs = [0]
    for s in sizes:
        offs.append(offs[-1] + s)

    nc = bacc.Bacc(
        "TRN2", target_bir_lowering=False, debug=False, num_devices=N_CORES
    )
    # tile-major flat layout: tile i occupies 6*P*F_i floats; within a tile,
    # partition p's 6*F_i floats are contiguous (one DMA descriptor per row).
    x = nc.dram_tensor("x", [6 * P * FREE], f32, kind="ExternalInput")
    out = nc.dram_tensor("out", [P, 32], f32, kind="ExternalOutput")
    xf = x.ap()

    def sb(name, shape, dtype):
        return nc.alloc_sbuf_tensor(name, list(shape), dtype).ap()

    inbuf = [sb(f"inb{b}", [P, 6 * Fmax], f32) for b in range(NB)]
    sqb = [sb(f"sqb{b}", [P, 6 * Fmax], bf16) for b in range(2)]
    m = sb("m", [P, 3 * Fmax], bf16)
    dxy = sb("dxy", [P, Fmax], bf16)
    dotb = [sb(f"dot{b}", [P, Fmax], bf16) for b in range(2)]
    pair = sb("pair", [P, 2 * Fmax], bf16)  # [oo1, tt1]
    oott = sb("oott", [P, 2 * Fmax], bf16)  # [oo, tt]
    prodb = [sb(f"prod{b}", [P, Fmax], bf16) for b in range(2)]
    cb = sb("c", [P, Fmax], bf16)
    c2b = [sb(f"c2{b}", [P, Fmax], bf16) for b in range(2)]
    numpb = [sb(f"nump{b}", [P, Fmax], bf16) for b in range(2)]
    r1b = [sb(f"r1{b}", [P, Fmax], bf16) for b in range(2)]
    r2b = [sb(f"r2{b}", [P, Fmax], bf16) for b in range(2)]
    g_all = sb("g_all", [P, FREE], bf16)
    t_scr = sb("t_scr", [P, FREE], bf16)
    asum = sb("asum", [P, 32], f32)
    warm = sb("warm", [P, 1], bf16)
    bias0 = sb("bias0", [P, 1], f32)
    bias1 = sb("bias1", [P, 1], f32)

    S_fin = nc.alloc_semaphore("s_fin")
    NQ = 4  # rotating per-tile DMA sems: exact per-tile completion
    S_dmaq = [nc.alloc_semaphore(f"s_dma{q}") for q in range(NQ)]
    S_dmo = nc.alloc_semaphore("s_dmo")
    S_bias = nc.alloc_semaphore("s_bias")

    def dma_wait(eng, i):
        eng.wait_ge(S_dmaq[i % NQ], _tot[i])
    S_vein = nc.alloc_semaphore("s_vein")  # bigmult done (input o,t read)
    S_prod = nc.alloc_semaphore("s_prod")
    S_c2 = nc.alloc_semaphore("s_c2")
    S_veg = nc.alloc_semaphore("s_veg")
    S_sq = nc.alloc_semaphore("s_sq")
    S_r1 = nc.alloc_semaphore("s_r1")
    S_r2 = nc.alloc_semaphore("s_r2")

    with nc.Block(no_gpsimd_drain=True) as block:

        # Ramp tiles (i < NB) are column-split across BOTH HWDGE rings
        # (sync k=0 + scalar k=1) for ~2x arrival rate at the pipeline head;
        # steady tiles load whole on sync's ring while scalar computes.
        SPLIT = min(NB, T)
        # cumulative inc expected on each sem slot when tile i completes
        _tot = {}
        slot_tot = [0] * NQ
        for _i in range(T):
            slot_tot[_i % NQ] += 32 if _i < SPLIT else 16
            _tot[_i] = slot_tot[_i % NQ]

        def issue_chunk(eng, i, k):
            F = sizes[i]
            tile = xf[6 * P * offs[i] : 6 * P * offs[i + 1]].rearrange(
                "(p f) -> p f", p=P
            )
            if i < SPLIT:
                lo, hi = 3 * k * F, 3 * (k + 1) * F
            else:
                lo, hi = 0, 6 * F
            eng.dma_start(
                out=inbuf[i % NB][:, lo:hi], in_=tile[:, lo:hi]
            ).then_inc(S_dmaq[i % NQ], 16)

        def issue_guard(eng, i):
            if i >= NB:
                eng.wait_ge(S_vein, i - NB + 1)
                eng.wait_ge(S_sq, i - NB + 1)

        @block.sync
        def _(sync):
            for i in range(T):
                issue_guard(sync, i)
                issue_chunk(sync, i, 0)
            sync.wait_ge(S_fin, 1)
            sync.dma_start(out=out.ap()[:, :], in_=asum[:, :]).then_inc(S_dmo, 16)
            sync.wait_ge(S_dmo, 16)



        # Software pipeline with lag: VE iter i does the front half of tile
        # i, then c/c2/nump of tile i-1 and g of tile i-2, so in steady state
        # it never waits on same-iteration ScalarE results.
        @block.vector
        def _(vector):
            vector.memset(bias0[:], 0.0).then_inc(S_bias)
            vector.memset(bias1[:], 1.0).then_inc(S_bias)
            for i in range(T + 2):
                h = i % 2
                hp = (i - 1) % 2
                hg = (i - 2) % 2
                if i < T:
                    F = sizes[i]
                    inb = inbuf[i % NB]
                    dma_wait(vector, i)
                    vector.tensor_tensor(
                        m[:, : 3 * F], inb[:, : 3 * F], inb[:, 3 * F : 6 * F],
                        OP.mult,
                    ).then_inc(S_vein)
                    vector.tensor_tensor(
                        dxy[:, :F], m[:, :F], m[:, F : 2 * F], OP.add
                    )
                    vector.tensor_tensor(
                        dotb[h][:, :F], dxy[:, :F], m[:, 2 * F : 3 * F], OP.add
                    )
                    vector.wait_ge(S_sq, i + 1)
                    sq6 = sqb[h][:, : 6 * F].rearrange("p (j f) -> p j f", j=6)
                    pr = pair[:, : 2 * F].rearrange("p (j f) -> p j f", j=2)
                    ot = oott[:, : 2 * F].rearrange("p (j f) -> p j f", j=2)
                    vector.tensor_tensor(
                        pr[:], sq6[:, 0:5:3, :], sq6[:, 1:6:3, :], OP.add
                    )
                    vector.tensor_tensor(ot[:], pr[:], sq6[:, 2:6:3, :], OP.add)
                    vector.tensor_tensor(
                        prodb[h][:, :F], ot[:, 0, :], ot[:, 1, :], OP.mult
                    ).then_inc(S_prod)
                if 1 <= i <= T:
                    F = sizes[i - 1]
                    vector.wait_ge(S_r1, i)
                    vector.tensor_tensor(
                        cb[:, :F], dotb[hp][:, :F], r1b[hp][:, :F], OP.mult
                    )
                    vector.tensor_tensor(
                        c2b[hp][:, :F], cb[:, :F], cb[:, :F], OP.mult
                    ).then_inc(S_c2)
                    # nump = min(c-1, 0) = -relu(1-c)
                    vector.tensor_scalar(
                        numpb[hp][:, :F], cb[:, :F], 1.0, 0.0, OP.subtract, OP.min
                    )
                if i >= 2:
                    F = sizes[i - 2]
                    vector.wait_ge(S_r2, i - 1)
                    vector.tensor_tensor(
                        g_all[:, offs[i - 2] : offs[i - 1]],
                        numpb[hg][:, :F], r2b[hg][:, :F], OP.mult,
                    ).then_inc(S_veg)

        @block.scalar
        def _(scalar):
            for j in range(min(NB, T)):
                issue_chunk(scalar, j, 1)
            # pin the abs_reciprocal_sqrt table set before any real work
            # (first activation in program order decides the first table
            # load; bias=warm itself avoids needing an initialized const)
            scalar.activation(
                warm[:], warm[:], AF.Abs_reciprocal_sqrt, bias=warm[:], scale=0.0
            )
            scalar.wait_ge(S_bias, 2)
            scalar.wait_ge(S_dmaq[0], 16)
            scalar.activation(
                sqb[0][:, : 6 * sizes[0]], inbuf[0][:, : 6 * sizes[0]], AF.Square,
                bias=bias0[:],
            ).then_inc(S_sq)
            for i in range(T):
                h = i % 2
                hp = (i - 1) % 2
                if i + 1 < T:
                    # issue sq[i+1] before r1[i] so VE's pair-adds for tile
                    # i+1 are never starved behind this iteration's r1/r2
                    hn = (i + 1) % 2
                    F1 = sizes[i + 1]
                    dma_wait(scalar, i + 1)
                    if i + 1 >= 2:
                        scalar.wait_ge(S_prod, i)
                    scalar.activation(
                        sqb[hn][:, : 6 * F1], inbuf[(i + 1) % NB][:, : 6 * F1],
                        AF.Square, bias=bias0[:],
                    ).then_inc(S_sq)
                F = sizes[i]
                scalar.wait_ge(S_prod, i + 1)
                scalar.activation(
                    r1b[h][:, :F], prodb[h][:, :F], AF.Abs_reciprocal_sqrt,
                    bias=bias0[:],
                ).then_inc(S_r1)

                if i >= 1:
                    Fp = sizes[i - 1]
                    scalar.wait_ge(S_c2, i)
                    scalar.activation(
                        r2b[hp][:, :Fp], c2b[hp][:, :Fp], AF.Abs_reciprocal_sqrt,
                        bias=bias1[:], scale=-1.0,
                    ).then_inc(S_r2)
            hp = (T - 1) % 2
            Fp = sizes[T - 1]
            scalar.wait_ge(S_c2, T)
            scalar.activation(
                r2b[hp][:, :Fp], c2b[hp][:, :Fp], AF.Abs_reciprocal_sqrt,
                bias=bias1[:], scale=-1.0,
            ).then_inc(S_r2)
            # dummy arctan: forces the sigmoid-set table load now, overlapped
            # with VE's final g multiplies
            scalar.activation(warm[:], warm[:], AF.Arctan, bias=bias0[:], scale=0.0)
            scalar.wait_ge(S_veg, T)
            scalar.activation(
                t_scr[:], g_all[:], AF.Arctan, bias=bias0[:],
                accum_out=asum[:, 0:1],
            ).then_inc(S_fin)

    nc.compile()
    _BUILD_CACHE[key] = nc
    return nc


def _build_nc():
    key = (N_TILES, tuple(SQ_ON_VE))
    if key in _BUILD_CACHE:
        return _BUILD_CACHE[key]

    from contextlib import ExitStack

    import concourse.bass as bass
    import concourse.tile as tile
    from concourse import bacc, mybir

    AF = mybir.ActivationFunctionType
    OP = mybir.AluOpType
    f32 = mybir.dt.float32
    bf16 = mybir.dt.bfloat16

    Ft = FREE // N_TILES

    nc = bacc.Bacc(
        "TRN2", target_bir_lowering=False, debug=False, num_devices=N_CORES
    )
    x = nc.dram_tensor("x", [6 * P, FREE], f32, kind="ExternalInput")
    out = nc.dram_tensor("out", [P, 1], f32, kind="ExternalOutput")

    with tile.TileContext(nc) as tc, ExitStack() as ctx:
        inp = ctx.enter_context(tc.tile_pool(name="inp", bufs=2))
        mid = ctx.enter_context(tc.tile_pool(name="mid", bufs=2))
        per = ctx.enter_context(tc.tile_pool(name="persist", bufs=1))

        g_all = per.tile([P, FREE], bf16)
        t_scr = per.tile([P, FREE], bf16)
        asum = per.tile([P, 1], f32)

        # [6*P, FREE] -> [P, 6, FREE]: partition-stride FREE, plane-stride P*FREE
        xa = x.ap().rearrange("(j p) f -> p j f", j=6)

        for i in range(N_TILES):
            sl = bass.ts(i, Ft)

            blk = inp.tile([P, 6, Ft], f32, tag="inblk")
            nc.sync.dma_start(blk[:], xa[:, :, sl])
            planes = [blk[:, j, :] for j in range(6)]
            ox, oy, oz, tx, ty, tz = planes

            def tt(name, a, b, op, dtype=bf16):
                t = mid.tile([P, Ft], dtype, tag=name)
                nc.vector.tensor_tensor(t[:], a[:], b[:], op)
                return t

            # dot = ox*tx + oy*ty + oz*tz  (mults f32->bf16, adds bf16)
            mx = tt("mx", ox, tx, OP.mult)
            my = tt("my", oy, ty, OP.mult)
            mz = tt("mz", oz, tz, OP.mult)
            dxy = tt("dxy", mx, my, OP.add)
            dot = tt("dot", dxy, mz, OP.add)

            # squares -> oo, tt
            sq = {}
            for name, pl in (
                ("xo", ox), ("yo", oy), ("zo", oz),
                ("xt", tx), ("yt", ty), ("zt", tz),
            ):
                s = mid.tile([P, Ft], bf16, tag="sq" + name)
                if name in SQ_ON_VE:
                    nc.vector.tensor_tensor(s[:], pl[:], pl[:], OP.mult)
                else:
                    nc.scalar.square(s[:], pl[:])
                sq[name] = s
            oo1 = tt("oo1", sq["xo"], sq["yo"], OP.add)
            oo = tt("oo", oo1, sq["zo"], OP.add)
            tt1 = tt("tt1", sq["xt"], sq["yt"], OP.add)
            ttn = tt("ttn", tt1, sq["zt"], OP.add)

            prod = tt("prod", oo, ttn, OP.mult)
            r1 = mid.tile([P, Ft], bf16, tag="r1")
            nc.scalar.activation(r1[:], prod[:], AF.Abs_reciprocal_sqrt)
            c = tt("c", dot, r1, OP.mult)
            c2 = tt("c2", c, c, OP.mult)
            # num_neg = min(c-1, 0) = -relu(1-c); sign is undone on the host
            nump = mid.tile([P, Ft], bf16, tag="nump")
            nc.vector.tensor_scalar(
                nump[:], c[:], 1.0, 0.0, OP.subtract, OP.min
            )
            r2 = mid.tile([P, Ft], bf16, tag="r2")
            nc.scalar.activation(
                r2[:], c2[:], AF.Abs_reciprocal_sqrt, bias=1.0, scale=-1.0
            )
            nc.vector.tensor_tensor(g_all[:, sl], nump[:], r2[:], OP.mult)

        # one arctan pass over the whole shard; accum_out = per-partition sum
        # (accumulates sum of arctan(-g) = -sum arctan(g))
        nc.scalar.activation(
            t_scr[:], g_all[:], AF.Arctan, accum_out=asum[:]
        )
        nc.sync.dma_start(out.ap()[:, :], asum[:])

    nc.compile()
    _BUILD_CACHE[key] = nc
    return nc


def _shard_inputs(outputs, targets):
    o = np.ascontiguousarray(np.asarray(outputs), dtype=np.float32).reshape(-1, 3)
    t = np.ascontiguousarray(np.asarray(targets), dtype=np.float32).reshape(-1, 3)
    T = N_TILES
    Ft = FREE // T
    in_maps = []
    for cidx in range(N_CORES):
        lo, hi = cidx * PER_CORE, (cidx + 1) * PER_CORE
        oc = o[lo:hi]
        tc_ = t[lo:hi]
        planes = np.empty((6, P, FREE), dtype=np.float32)
        for k in range(3):
            planes[k] = oc[:, k].reshape(P, FREE)
            planes[3 + k] = tc_[:, k].reshape(P, FREE)
        if USE_RAW:
            # tile-major flat: per tile i, [P, 6, F_i] row-contiguous
            blocks = []
            off = 0
            for F in TILE_SIZES:
                blk = planes[:, :, off : off + F]  # [6, P, F]
                blocks.append(
                    np.ascontiguousarray(blk.transpose(1, 0, 2)).reshape(-1)
                )
                off += F
            arr = np.concatenate(blocks)
        else:
            arr = planes.reshape(6 * P, FREE)
        in_maps.append({"x": arr})
    return in_maps


LAST_RESULT = None


def kernel(outputs, targets):
    global LAST_RESULT
    import os

    from concourse.bass_utils import run_bass_kernel_spmd

    nc = _build_nc_raw() if USE_RAW else _build_nc()
    in_maps = _shard_inputs(outputs, targets)
    trace = bool(os.environ.get("ANGLE_KERNEL_TRACE"))
    res = run_bass_kernel_spmd(
        nc, in_maps, core_ids=list(range(N_CORES)), trace=trace
    )
    LAST_RESULT = res
    total = 0.0
    for rmap in res.results:
        o = np.asarray(rmap["out"], dtype=np.float64)
        if USE_RAW:
            o = o[:, 0]  # cols 1.. are DMA padding
        total += o.sum()
    # device accumulates sum(arctan(-g)); theta = 2*arctan(g)
    mean = -2.0 * total / R_TOTAL
    return np.float32(mean)


# revision 24
# speedup vs baseline: 1.0728x; 1.0727x over previous
"""AngleLoss distributed Trainium2 kernel.

mean(arccos(dot(o,t) / (|o||t|))) over 2,097,152 rows of 3-vectors,
data-parallel over 8 NeuronCores (no collective needed: each core returns
per-partition partial sums, host adds 1024 floats).

Math per row (division- and sign-free):
    dot  = sum o*t ; oo = sum o^2 ; tt = sum t^2      (bf16 compute)
    c    = dot * absrsqrt(oo*tt)                      # cos(theta)
    nump = relu(1 - c)                                # clamped 1-c
    r2   = absrsqrt(|1 - c^2|)
    g    = nump * r2        # = sqrt((1-c)/(1+c)) = tan(theta/2) in [0,inf)
    theta = 2*arctan(g)     # cayman arctan table covers [0,inf), inf->pi/2
The Arctan pass uses accum_out for the per-partition reduction.

Layout: host pre-shards rows 8 ways and stores each shard component-planar,
tile-major: tile i holds [128 partitions x (6 planes * F_i)] with each
partition's 6*F_i floats contiguous (large DMA descriptors). Both HWDGE
rings (sync + scalar) issue loads, alternating tiles.
"""

import sys

import numpy as np

if "/opt/trn_rl_repo" not in sys.path:
    sys.path.insert(0, "/opt/trn_rl_repo")

N_CORES = 8
R_TOTAL = 256 * 8192  # 2097152 rows
PER_CORE = R_TOTAL // N_CORES  # 262144
P = 128
FREE = PER_CORE // P  # 2048

TILE_SIZES = (128, 256, 384, 512, 512, 256)
N_INBUF = 4
assert sum(TILE_SIZES) == FREE

_BUILD_CACHE = {}


def _build_nc():
    key = (TILE_SIZES, N_INBUF)
    if key in _BUILD_CACHE:
        return _BUILD_CACHE[key]

    from concourse import bacc, mybir

    AF = mybir.ActivationFunctionType
    OP = mybir.AluOpType
    f32 = mybir.dt.float32
    bf16 = mybir.dt.bfloat16

    sizes = list(TILE_SIZES)
    T = len(sizes)
    NB = N_INBUF
    NQ = 4
    Fmax = max(sizes)
    offs = [0]
    for s in sizes:
        offs.append(offs[-1] + s)
    # cumulative value of the tile's rotating DMA sem when it completes
    tot = {}
    slot_tot = [0] * NQ
    for i in range(T):
        slot_tot[i % NQ] += 16
        tot[i] = slot_tot[i % NQ]

    nc = bacc.Bacc(
        "TRN2", target_bir_lowering=False, debug=False, num_devices=N_CORES
    )
    x = nc.dram_tensor("x", [6 * P * FREE], f32, kind="ExternalInput")
    out = nc.dram_tensor("out", [P, 32], f32, kind="ExternalOutput")
    xf = x.ap()

    def sb(name, shape, dtype):
        return nc.alloc_sbuf_tensor(name, list(shape), dtype).ap()

    inbuf = [sb(f"inb{b}", [P, 6 * Fmax], f32) for b in range(NB)]
    sqb = [sb(f"sqb{b}", [P, 6 * Fmax], bf16) for b in range(2)]
    m = sb("m", [P, 3 * Fmax], bf16)
    dxy = sb("dxy", [P, Fmax], bf16)
    dotb = [sb(f"dot{b}", [P, Fmax], bf16) for b in range(2)]
    pair = sb("pair", [P, 2 * Fmax], bf16)  # [oo1, tt1]
    oott = sb("oott", [P, 2 * Fmax], bf16)  # [oo, tt]
    prodb = [sb(f"prod{b}", [P, Fmax], bf16) for b in range(2)]
    cb = [sb(f"c{b}", [P, Fmax], bf16) for b in range(2)]
    c2s = sb("c2s", [P, Fmax], bf16)
    numpb = [sb(f"nump{b}", [P, Fmax], bf16) for b in range(2)]
    r1b = [sb(f"r1{b}", [P, Fmax], bf16) for b in range(2)]
    r2b = [sb(f"r2{b}", [P, Fmax], bf16) for b in range(2)]
    g_all = sb("g_all", [P, FREE], bf16)
    t_scr = sb("t_scr", [P, FREE], bf16)
    asum = sb("asum", [P, 32], f32)
    warm = sb("warm", [P, 1], bf16)
    bias0 = sb("bias0", [P, 1], f32)
    bias1 = sb("bias1", [P, 1], f32)

    S_dmaq = [nc.alloc_semaphore(f"s_dma{q}") for q in range(NQ)]
    S_dmo = nc.alloc_semaphore("s_dmo")
    S_bias = nc.alloc_semaphore("s_bias")
    S_vein = nc.alloc_semaphore("s_vein")  # 1/tile: bigmult read inputs
    S_prod = nc.alloc_semaphore("s_prod")  # 1/tile
    S_c2 = nc.alloc_semaphore("s_c2")  # 1/tile: c written
    S_veg = nc.alloc_semaphore("s_veg")  # 1/tile: g written
    S_sq = nc.alloc_semaphore("s_sq")  # 1/tile
    S_r1 = nc.alloc_semaphore("s_r1")  # 1/tile
    S_r2 = nc.alloc_semaphore("s_r2")  # 1/tile
    S_fin = nc.alloc_semaphore("s_fin")

    def dma_wait(eng, i):
        eng.wait_ge(S_dmaq[i % NQ], tot[i])

    with nc.Block(no_gpsimd_drain=True) as block:

        def issue_in_dma(eng, i):
            tile = xf[6 * P * offs[i] : 6 * P * offs[i + 1]].rearrange(
                "(p f) -> p f", p=P
            )
            eng.dma_start(
                out=inbuf[i % NB][:, : 6 * sizes[i]], in_=tile
            ).then_inc(S_dmaq[i % NQ], 16)

        def issue_guard(eng, i):
            if i >= NB:
                eng.wait_ge(S_vein, i - NB + 1)
                eng.wait_ge(S_sq, i - NB + 1)

        @block.sync
        def _(sync):
            # even tiles on sync's HWDGE ring (odd tiles go via ScalarE's)
            for i in range(0, T, 2):
                issue_guard(sync, i)
                issue_in_dma(sync, i)
            sync.wait_ge(S_fin, 1)
            sync.dma_start(out=out.ap()[:, :], in_=asum[:, :]).then_inc(
                S_dmo, 16
            )
            sync.wait_ge(S_dmo, 16)

        # Software pipeline with lag: VE iter i runs the front half of tile
        # i, then c of tile i-1, then g of tile i-2, so in steady state it
        # never waits on same-iteration ScalarE results.
        @block.vector
        def _(vector):
            vector.memset(bias0[:], 0.0).then_inc(S_bias)
            vector.memset(bias1[:], 1.0).then_inc(S_bias)
            for i in range(T + 2):
                h = i % 2
                hp = (i - 1) % 2
                hg = (i - 2) % 2
                if i < T:
                    F = sizes[i]
                    inb = inbuf[i % NB]
                    dma_wait(vector, i)
                    vector.tensor_tensor(
                        m[:, : 3 * F], inb[:, : 3 * F], inb[:, 3 * F : 6 * F],
                        OP.mult,
                    ).then_inc(S_vein)
                    vector.tensor_tensor(
                        dxy[:, :F], m[:, :F], m[:, F : 2 * F], OP.add
                    )
                    vector.tensor_tensor(
                        dotb[h][:, :F], dxy[:, :F], m[:, 2 * F : 3 * F], OP.add
                    )
                    vector.wait_ge(S_sq, i + 1)
                    sq6 = sqb[h][:, : 6 * F].rearrange("p (j f) -> p j f", j=6)
                    pr = pair[:, : 2 * F].rearrange("p (j f) -> p j f", j=2)
                    ot = oott[:, : 2 * F].rearrange("p (j f) -> p j f", j=2)
                    vector.tensor_tensor(
                        pr[:], sq6[:, 0:5:3, :], sq6[:, 1:6:3, :], OP.add
                    )
                    vector.tensor_tensor(
                        ot[:], pr[:], sq6[:, 2:6:3, :], OP.add
                    )
                    vector.tensor_tensor(
                        prodb[h][:, :F], ot[:, 0, :], ot[:, 1, :], OP.mult
                    ).then_inc(S_prod)
                if 1 <= i <= T:
                    F = sizes[i - 1]
                    vector.wait_ge(S_r1, i)
                    vector.tensor_tensor(
                        cb[hp][:, :F], dotb[hp][:, :F], r1b[hp][:, :F],
                        OP.mult,
                    ).then_inc(S_c2)
                if i >= 2:
                    F = sizes[i - 2]
                    vector.wait_ge(S_r2, i - 1)
                    vector.tensor_tensor(
                        g_all[:, offs[i - 2] : offs[i - 1]],
                        numpb[hg][:, :F], r2b[hg][:, :F], OP.mult,
                    ).then_inc(S_veg)

        @block.scalar
        def _(scalar):
            def triple(i):
                # c2/nump/r2 for tile i (reads cb written by VE)
                hh = i % 2
                F = sizes[i]
                scalar.wait_ge(S_c2, i + 1)
                scalar.activation(
                    c2s[:, :F], cb[hh][:, :F], AF.Square, bias=bias0[:]
                )
                scalar.activation(
                    numpb[hh][:, :F], cb[hh][:, :F], AF.Relu,
                    bias=bias1[:], scale=-1.0,
                )
                scalar.activation(
                    r2b[hh][:, :F], c2s[:, :F], AF.Abs_reciprocal_sqrt,
                    bias=bias1[:], scale=-1.0,
                ).then_inc(S_r2)

            # odd ramp tiles are issued on ScalarE's HWDGE ring
            for j in range(1, min(NB, T), 2):
                issue_in_dma(scalar, j)
            # first activation in program order pins the absrsqrt table set;
            # bias=warm itself avoids needing an initialized constant
            scalar.activation(
                warm[:], warm[:], AF.Abs_reciprocal_sqrt, bias=warm[:],
                scale=0.0,
            )
            scalar.wait_ge(S_bias, 2)
            dma_wait(scalar, 0)
            scalar.activation(
                sqb[0][:, : 6 * sizes[0]], inbuf[0][:, : 6 * sizes[0]],
                AF.Square, bias=bias0[:],
            ).then_inc(S_sq)
            for i in range(T):
                h = i % 2
                if i + 1 < T:
                    # sq[i+1] ahead of r1[i] so VE's pair-adds for tile i+1
                    # are never starved behind this iteration's r1/r2
                    hn = (i + 1) % 2
                    F1 = sizes[i + 1]
                    dma_wait(scalar, i + 1)
                    if i + 1 >= 2:
                        # sqb[hn] free: tile i-1's pair-adds are done
                        scalar.wait_ge(S_prod, i)
                    scalar.activation(
                        sqb[hn][:, : 6 * F1],
                        inbuf[(i + 1) % NB][:, : 6 * F1],
                        AF.Square, bias=bias0[:],
                    ).then_inc(S_sq)
                F = sizes[i]
                scalar.wait_ge(S_prod, i + 1)
                scalar.activation(
                    r1b[h][:, :F], prodb[h][:, :F], AF.Abs_reciprocal_sqrt,
                    bias=bias0[:],
                ).then_inc(S_r1)
                if i + NB < T and (i + NB) % 2 == 1:
                    # inbuf[(i+NB)%NB] free: implied by S_prod>=i+1 (VE's
                    # bigmult of tile i) + own sq[i] earlier
                    issue_in_dma(scalar, i + NB)
                if i >= 1:
                    triple(i - 1)
            triple(T - 1)
            # dummy arctan: forces the sigmoid-set table load now,
            # overlapping VE's final g multiplies
            scalar.activation(
                warm[:], warm[:], AF.Arctan, bias=bias0[:], scale=0.0
            )
            scalar.wait_ge(S_veg, T)
            scalar.activation(
                t_scr[:], g_all[:], AF.Arctan, bias=bias0[:],
                accum_out=asum[:, 0:1],
            ).then_inc(S_fin)

    nc.compile()
    _BUILD_CACHE[key] = nc
    return nc


def _shard_inputs(outputs, targets):
    o = np.ascontiguousarray(np.asarray(outputs), dtype=np.float32).reshape(-1, 3)
    t = np.ascontiguousarray(np.asarray(targets), dtype=np.float32).reshape(-1, 3)
    in_maps = []
    for cidx in range(N_CORES):
        lo, hi = cidx * PER_CORE, (cidx + 1) * PER_CORE
        oc = o[lo:hi]
        tc_ = t[lo:hi]
        planes = np.empty((6, P, FREE), dtype=np.float32)
        for k in range(3):
            planes[k] = oc[:, k].reshape(P, FREE)
            planes[3 + k] = tc_[:, k].reshape(P, FREE)
        # tile-major flat: per tile, [P, 6, F_i] with rows contiguous
        blocks = []
        off = 0
        for F in TILE_SIZES:
            blk = planes[:, :, off : off + F]  # [6, P, F]
            blocks.append(
                np.ascontiguousarray(blk.transpose(1, 0, 2)).reshape(-1)
            )
            off += F
        in_maps.append({"x": np.concatenate(blocks)})
    return in_maps


LAST_RESULT = None


def kernel(outputs, targets):
    global LAST_RESULT
    import os

    from concourse.bass_utils import run_bass_kernel_spmd

    nc = _build_nc()
    in_maps = _shard_inputs(outputs, targets)
    trace = bool(os.environ.get("ANGLE_KERNEL_TRACE"))
    res = run_bass_kernel_spmd(
        nc, in_maps, core_ids=list(range(N_CORES)), trace=trace
    )
    LAST_RESULT = res
    total = 0.0
    for rmap in res.results:
        total += np.asarray(rmap["out"], dtype=np.float64)[:, 0].sum()
    mean = 2.0 * total / R_TOTAL
    return np.float32(mean)
